# revision 13
# baseline (speedup 1.0000x reference)
"""Fused QKV-projection + multi-head attention kernel for Trainium2.

Problem: x [2, 2048, 1024] fp32; W_qkv [1024, 3072]; b_qkv [3072].
  qkv = x @ W_qkv + b; 16 heads of 64; scores = q k^T / 8; softmax; out = attn @ v.

Sharding: 8 cores = 2 (batch) x 4 (head groups of 4 heads). Each core is fully
independent (no collectives): it computes the projection for its batch restricted
to its 4 heads' q/k/v columns, then attention for those heads.

Per-core device layout tricks:
  - host feeds x^T (with an appended ones-row) so both projection matmuls can
    consume it directly; the bias is folded into the weights as an extra
    contraction row (W_aug row 1024 = bias, x^T_aug row 1024 = 1).
  - q/k are produced TRANSPOSED (qT/kT [64, S]) so scores^T = kT.T @ qT needs
    no transposes; softmax runs as a single fused exp (no max subtraction --
    scores are O(+-8) so exp stays in fp16 range).
  - V is produced in natural [S, 64] layout per head with an extra constant-1
    column (generated by the same augmented matmul): accumulating
    O^T_aug = V_aug.T @ exp(S^T) yields the attention output AND the softmax
    denominators in one matmul stream.
  - O^T [65, 512] blocks are transposed back via the PE transpose path and
    normalized by the per-row reciprocal denominator on the way to SBUF.
  - matmul operands are fp16 (measured end-to-end rel err ~1.1e-3 vs the fp32
    reference; fp32-strict matmuls are 4x slower and fp32r hits a walrus
    sync-wait limit). PSUM accumulation is fp32; the softmax denominators
    (~5e6) overflow fp16, so the O^T staging/transpose stays fp32.

Sync architecture: walrus rejects Matmult instructions carrying more than one
semaphore wait ("Too many sync wait commands"). All multi-source dependencies
are therefore funneled through PE nop "gates" (which accept many waits), and
scores/PV matmuls are explicitly interleaved so that by the time a matmul
needs an ACT/DVE-produced tile, an earlier PE instruction has already waited
on a later tick of that same semaphore.
"""

import sys

if "/opt/trn_rl_repo" not in sys.path:
    sys.path.insert(0, "/opt/trn_rl_repo")

import numpy as np
from contextlib import ExitStack

B, S, D = 2, 2048, 1024
H, Dh = 16, 64
HL = 4          # heads per core
GW = HL * Dh    # 256 output cols per core
VW = HL * 65    # V_aug width: per head [v (64) | ones (1)]
KC = 1025       # augmented contraction (1024 + bias row)
NST = S // 128  # 16 s-tiles
NQB = S // 512  # 4 q blocks

_CACHE = {}


def _build_nc():
    import concourse.bass as bass
    import concourse.mybir as mybir
    import concourse.tile as tile
    from concourse.tile import add_dep_helper

    f32 = mybir.dt.float32
    f16 = mybir.dt.float16
    Exp = mybir.ActivationFunctionType.Exp

    nc = bass.Bass()
    xT = nc.dram_tensor("xT", [KC, S], f16, kind="ExternalInput")
    wqk = nc.dram_tensor("wqk", [KC, 512], f16, kind="ExternalInput")
    wv = nc.dram_tensor("wv", [KC, VW], f16, kind="ExternalInput")
    out = nc.dram_tensor("out", [S, GW], f32, kind="ExternalOutput")

    def chunks():
        for d in range(9):
            yield d, (128 if d < 8 else 1)

    def engine_gate(eng, hint, producers):
        """Chain of nops on `eng`, each waiting on ONE producer, so no single
        instruction (nops included) carries more than one semaphore wait.
        Returns the last nop; order gated instructions after it."""
        g = None
        for i, p in enumerate(producers):
            n = eng.nop(nofuse=True, hint=f"{hint}_{i}")
            add_dep_helper(n.ins, p.ins, reason=f"{hint} pre-wait")
            if g is not None:
                add_dep_helper(n.ins, g.ins, reason=f"{hint} chain")
            g = n
        if g is None:
            g = eng.nop(nofuse=True, hint=hint)
        return g

    def pe_gate(hint, producers):
        return engine_gate(nc.tensor, hint, producers)

    def after(a, b, why="order"):
        add_dep_helper(a.ins, b.ins, reason=why)

    with tile.TileContext(nc) as tc, ExitStack() as ctx:
        persist = ctx.enter_context(tc.tile_pool(name="persist", bufs=1))
        qT = [persist.tile([64, S], f16, name=f"qT{h}", tag=f"qT{h}") for h in range(HL)]
        kT = [persist.tile([64, S], f16, name=f"kT{h}", tag=f"kT{h}") for h in range(HL)]
        V = [persist.tile([128, VW], f16, name=f"V{t}", tag=f"V{t}") for t in range(NST)]
        ost = [persist.tile([128, GW], f32, name=f"ost{t}", tag=f"ost{t}") for t in range(NST)]
        ident = persist.tile([128, 128], f32, name="ident", tag="ident")
        id_i1 = nc.gpsimd.memset(ident, 0.0)
        id_i2 = nc.gpsimd.affine_select(
            out=ident, in_=ident, compare_op=mybir.AluOpType.not_equal,
            fill=1.0, base=0, pattern=[[-1, 128]], channel_multiplier=1)

        phase1_copies = []  # all DVE psum->sbuf copies (phase-2 gate deps)

        # ---------------- Phase 1: qkv projection ----------------
        with tc.tile_pool(name="wpool", bufs=1) as wpool, \
             tc.tile_pool(name="xpool", bufs=1) as xpool, \
             tc.tile_pool(name="psA", bufs=4, space="PSUM") as psA, \
             tc.tile_pool(name="psV", bufs=2, space="PSUM") as psV:
            wqk_sb, wv_sb, wdma = [], [], []
            for d, p in chunks():
                twq = wpool.tile([p, 512], f16, name=f"wq{d}", tag=f"wq{d}")
                wdma.append(nc.sync.dma_start(out=twq, in_=wqk[d * 128:d * 128 + p, :]))
                twv = wpool.tile([p, VW], f16, name=f"wv{d}", tag=f"wv{d}")
                wdma.append(nc.sync.dma_start(out=twv, in_=wv[d * 128:d * 128 + p, :]))
                wqk_sb.append(twq)
                wv_sb.append(twv)

            qk_copies = []   # per qkT-group list of DVE copies
            v_copies = []    # per V-group copy
            all_dmas = list(wdma)
            for sh in range(2):
                xT_sb, xdma = [], []
                for d, p in chunks():
                    t = xpool.tile([p, 1024], f16, name=f"x{sh}_{d}", tag=f"x{d}")
                    xdma.append(nc.sync.dma_start(
                        out=t, in_=xT[d * 128:d * 128 + p, sh * 1024:(sh + 1) * 1024]))
                    xT_sb.append(t)
                all_dmas.extend(xdma)
                gate_dma = pe_gate(f"gate_dma{sh}", xdma + (wdma if sh == 0 else []))

                # qT/kT: out rows = qkv columns (M-tiles), free = sequence
                # mt 0: q heads 0,1 | mt 1: q heads 2,3 | mt 2: k heads 0,1 | mt 3: k
                for mt in range(4):
                    g = len(qk_copies)
                    gdeps = qk_copies[g - 2] if g >= 2 else []
                    gate = pe_gate(f"gate_qk{sh}_{mt}", gdeps) if gdeps else gate_dma
                    ps = [psA.tile([128, 512], f32, name=f"psA{sh}_{mt}_{j}", tag="psA")
                          for j in range(2)]
                    for d, p in chunks():
                        lhsT = wqk_sb[d][:, mt * 128:(mt + 1) * 128]
                        for j in range(2):
                            mm = nc.tensor.matmul(
                                ps[j], lhsT,
                                xT_sb[d][:, j * 512:(j + 1) * 512],
                                start=(d == 0), stop=(d == 8))
                            after(mm, gate)
                            after(mm, gate_dma)
                    grp = []
                    for j in range(2):
                        qb = sh * 2 + j
                        for half in range(2):
                            hloc = (mt % 2) * 2 + half
                            dst = (qT if mt < 2 else kT)[hloc]
                            cp = nc.vector.tensor_copy(
                                dst[:, qb * 512:(qb + 1) * 512],
                                ps[j][half * 64:(half + 1) * 64, :])
                            grp.append(cp)
                    qk_copies.append(grp)
                    phase1_copies.extend(grp)

                # V_aug: natural layout [s-tile, 4*(64+1)]
                for stl in range(8):
                    st = sh * 8 + stl
                    gdeps = [v_copies[st - 2]] if st >= 2 else []
                    gate = pe_gate(f"gate_v{st}", gdeps) if gdeps else gate_dma
                    psv = psV.tile([128, VW], f32, name=f"psV{st}", tag="psV")
                    for d, p in chunks():
                        mm = nc.tensor.matmul(
                            psv,
                            xT_sb[d][:, stl * 128:(stl + 1) * 128],
                            wv_sb[d],
                            start=(d == 0), stop=(d == 8))
                        after(mm, gate)
                        after(mm, gate_dma)
                    cp = nc.vector.tensor_copy(V[st], psv)
                    v_copies.append(cp)
                    phase1_copies.append(cp)

        # ---------------- Phase 2: attention ----------------
        with tc.tile_pool(name="expp", bufs=8) as expp, \
             tc.tile_pool(name="normp", bufs=4) as normp, \
             tc.tile_pool(name="psS", bufs=2, space="PSUM") as psS, \
             tc.tile_pool(name="psO", bufs=2, space="PSUM") as psO, \
             tc.tile_pool(name="psT", bufs=2, space="PSUM") as psT:
            act_gate = engine_gate(
                nc.scalar, "act_gate", all_dmas + phase1_copies + [id_i1, id_i2])
            dve_gate = engine_gate(nc.vector, "dve_gate", all_dmas)
            prev_exps = phase1_copies + [id_i1, id_i2]
            prev_ot = None
            for h in range(HL):
                for qb in range(NQB):
                    gate = pe_gate(
                        f"gate_{h}_{qb}",
                        prev_exps + ([prev_ot] if prev_ot is not None else []))

                    # scores^T in sk-tile pairs + fused exp
                    ets, exps, smms = [], [], []
                    for sp in range(8):
                        ps = psS.tile([128, 1024], f32, name=f"s{h}_{qb}_{sp}", tag="psS")
                        pair = []
                        for half in range(2):
                            st = sp * 2 + half
                            mm = nc.tensor.matmul(
                                ps[:, half * 512:(half + 1) * 512],
                                kT[h][:, st * 128:(st + 1) * 128],
                                qT[h][:, qb * 512:(qb + 1) * 512],
                                start=True, stop=True)
                            after(mm, gate)
                            pair.append(mm)
                        smms.append(pair)
                        et = expp.tile([128, 1024], f16, name=f"e{h}_{qb}_{sp}", tag="expS")
                        ea = nc.scalar.activation(et, ps, Exp, scale=0.125)
                        if h == 0 and qb == 0:
                            after(ea, act_gate)
                        exps.append(ea)
                        ets.append(et)

                    # O^T_aug = V_aug.T @ exp(S^T): rows 0..63 = out, row 64 = denom
                    po = psO.tile([65, 512], f32, name=f"po{h}_{qb}", tag="psO")
                    pvs = []
                    for st in range(NST):
                        mm = nc.tensor.matmul(
                            po,
                            V[st][:, h * 65:(h + 1) * 65],
                            ets[st // 2][:, (st % 2) * 512:(st % 2 + 1) * 512],
                            start=(st == 0), stop=(st == NST - 1))
                        after(mm, gate)
                        pvs.append(mm)
                    # interleave so scores(sp) runs only after PV consumed the
                    # psS slot's previous occupant (covers the ACT release wait)
                    for sp in range(2, 8):
                        for mm in smms[sp]:
                            after(mm, pvs[2 * (sp - 2) + 1], "psS slot covered by PV")

                    ot = normp.tile([65, 512], f32, name=f"ot{h}_{qb}", tag="ot")
                    prev_ot = nc.vector.tensor_copy(ot, po)
                    if h == 0 and qb == 0:
                        after(prev_ot, dve_gate)

                    # transpose back to [sq, 65], normalize, stage output
                    for c in range(4):
                        pt = psT.tile([128, 65], f32, name=f"pt{h}_{qb}_{c}", tag="psT")
                        tr = nc.tensor.transpose(pt, ot[:, c * 128:(c + 1) * 128],
                                                 ident[:65, :65])
                        after(tr, gate)
                        rec = normp.tile([128, 1], f32, name=f"rec{h}_{qb}_{c}", tag="rec")
                        nc.vector.reciprocal(rec, pt[:, 64:65])
                        qt = qb * 4 + c
                        nc.vector.tensor_scalar_mul(
                            ost[qt][:, h * Dh:(h + 1) * Dh], pt[:, :Dh], rec)
                        if h == HL - 1:
                            nc.sync.dma_start(
                                out=out[qt * 128:(qt + 1) * 128, :], in_=ost[qt])
                    prev_exps = exps
    return nc


def _relax_waits(nc):
    """Walrus rejects instructions carrying more than ~1 embedded semaphore
    wait ("Too many sync wait commands"). Strip waits that are provably
    redundant. Soundness (this kernel is fully unrolled: no loops, no sem
    resets, all sems monotone):
      R1: a PE instruction never needs a wait on PE's own completion
          semaphore: PE executes in order, never reads its own output
          (no PSUM read port), and drains (PSUM writes) are in order.
      R2: a wait (sem >= v) is redundant if an earlier instruction on the
          same engine already waits (sem >= v' >= v): the per-engine
          sequencer processes waits in stream order.
    Returns the number of instructions still carrying >1 ge-waits."""
    # Only PE: it never reads its own writes (no PSUM read port), and its
    # in-order drain sequences PSUM WAW. DVE/ACT have deep non-interlocked
    # pipelines -- their self-waits guard real RAW hazards.
    own_sem = {"PE": "PE_"}
    observed = {}  # (engine, sem id) -> max value waited
    remaining = 0
    for fn in nc.m.functions:
        for blk in fn.blocks:
            for inst in blk.instructions:
                si = getattr(inst, "sync_info", None)
                if si is None or not si.on_wait:
                    continue
                eng = str(inst.engine).split(".")[-1]
                pfx = own_sem.get(eng)
                keep, nge = [], 0
                for w in si.on_wait:
                    if w.sync_type != "semaphore" or w.wait_mode != "sem-ge-imm" \
                            or w.wait_reg is not None \
                            or w.ant_name.startswith("barrier_"):
                        # barrier sems are decremented (non-monotone): hands off
                        keep.append(w)
                        continue
                    if pfx is not None and w.ant_name.startswith(pfx):
                        continue  # R1
                    k = (eng, w.id)
                    if observed.get(k, -1) >= w.wait_value:
                        continue  # R2
                    observed[k] = w.wait_value
                    keep.append(w)
                    nge += 1
                if nge > 1:
                    remaining += 1
                if len(keep) != len(si.on_wait):
                    si.on_wait = keep
                    inst.sync_info = si
    return remaining


def _split_multi_waits(nc):
    """Any instruction still carrying >1 ge-waits after relaxation gets its
    excess waits hoisted onto same-engine NoOps inserted right before it
    (a sequence of single-wait instructions is semantically identical to one
    multi-wait instruction on an in-order sequencer)."""
    import bass_rust

    def wkey(w):
        return (w.id, w.wait_value, w.wait_mode)

    plan = {}
    for fn in nc.m.functions:
        for blk in fn.blocks:
            for inst in blk.instructions:
                si = getattr(inst, "sync_info", None)
                if si is None or not si.on_wait:
                    continue
                ow = list(si.on_wait)
                ge = [w for w in ow
                      if w.sync_type == "semaphore" and w.wait_mode == "sem-ge-imm"
                      and w.wait_reg is None
                      and not w.ant_name.startswith("barrier_")]
                if len(ge) <= 1:
                    continue
                hoist = ge[1:]
                hkeys = {wkey(w) for w in hoist}
                nops = []
                for w in hoist:
                    nb = nc.engines[inst.engine].nop(nofuse=True, hint="wait_split")
                    ni = nb.ins
                    ni.sync_info = bass_rust.SyncInfo(on_wait=[w], on_update=[])
                    nops.append(ni)
                plan[inst.name] = nops
                si.on_wait = [w for w in ow if wkey(w) not in hkeys
                              or (w.sync_type, w.wait_mode) != ("semaphore", "sem-ge-imm")]
                inst.sync_info = si
    if not plan:
        return 0
    created = {n.name for nops in plan.values() for n in nops}
    for fn in nc.m.functions:
        for blk in fn.blocks:
            cur = list(blk.instructions)
            new = []
            for i in cur:
                if i.name in created:
                    continue
                if i.name in plan:
                    new.extend(plan[i.name])
                new.append(i)
            blk.instructions = new
    return len(plan)


def get_nc():
    if "nc" not in _CACHE:
        nc = _build_nc()
        _relax_waits(nc)
        _split_multi_waits(nc)
        _CACHE["nc"] = nc
    return _CACHE["nc"]


def prep_inputs(x, W_qkv, b_qkv):
    """Host-side sharding: returns the 8 per-core input maps."""
    x = np.asarray(x, dtype=np.float32)
    W_qkv = np.asarray(W_qkv, dtype=np.float32)
    b_qkv = np.asarray(b_qkv, dtype=np.float32)
    ones = np.ones((1, S), np.float32)
    in_maps = []
    for c in range(8):
        b, g = divmod(c, 4)
        xTm = np.concatenate([np.ascontiguousarray(x[b].T), ones], axis=0).astype(np.float16)
        heads = list(range(HL * g, HL * g + HL))
        cols = np.concatenate([np.arange(h * Dh, (h + 1) * Dh) for h in heads])
        wqk_m = np.empty((KC, 512), np.float16)
        wqk_m[:D, :256] = W_qkv[:, cols]
        wqk_m[D, :256] = b_qkv[cols]
        wqk_m[:D, 256:] = W_qkv[:, D + cols]
        wqk_m[D, 256:] = b_qkv[D + cols]
        wv_m = np.zeros((KC, VW), np.float16)
        for i, h in enumerate(heads):
            vcols = 2 * D + h * Dh
            wv_m[:D, i * 65:i * 65 + 64] = W_qkv[:, vcols:vcols + Dh]
            wv_m[D, i * 65:i * 65 + 64] = b_qkv[vcols:vcols + Dh]
            wv_m[D, i * 65 + 64] = 1.0  # generates the constant-1 denom column
        in_maps.append({"xT": xTm, "wqk": wqk_m, "wv": wv_m})
    return in_maps


def assemble_output(results):
    out = np.empty((B, S, D), np.float32)
    for c in range(8):
        b, g = divmod(c, 4)
        out[b, :, g * GW:(g + 1) * GW] = results[c]["out"]
    return out


def kernel(x, W_qkv, b_qkv):
    from concourse.bass_utils import run_bass_kernel_spmd

    nc = get_nc()
    in_maps = prep_inputs(x, W_qkv, b_qkv)
    res = run_bass_kernel_spmd(nc, in_maps, list(range(8)))
    return assemble_output(res.results)
